# revision 1
# baseline (speedup 1.0000x reference)
"""Viterbi decode kernel builder for TRN2 (Bass/Tile).

Layout (per core, B_loc=16 batch rows):
  partition p = b*8 + ch   (b in [0,16), ch in [0,8));  cur = ch*16 + cl
  TRW  [128, 16*128] f32 : TRW[b*8+ch, cl*128+q] = trans[q, ch*16+cl]
  POT  [128, T*16]   f32 : POT[b*8+ch, t*16+cl]  = pot[b, t, ch*16+cl]
  AHIST DRAM [128, T*16] : same layout as POT, holds alpha_t (partition-major;
                           alpha_t[b, cur] = AHIST[b*8+ch, t*16+cl])
  G1/G2/G3 [128,128] bf16: exact 3-term split of transT (transT[c,q]=trans[q,c])
  IOTA [128, 16] f32     : partition index replicated
  ONES [1, 128]  f32
Forward per t: 16x TTR (scores add + max over prev) -> M[128,16];
  alpha_next = M + pot_t -> stage; 8x stream_shuffle -> ALPHA_P[128,128];
  stage DMA'd to AHIST per unroll-group.  Single For_i loop.
Backward per t (descending, single For_i): onehot(tag) via PE broadcast +
  iota compare; trans column via 3x bf16 matmuls; TTR add+max; running-max
  scan; count(run<max) = first-index argmax = next tag.
"""
from contextlib import ExitStack

import numpy as np
import ml_dtypes

import concourse.bass as bass
import concourse.tile as tile
from concourse import mybir


def legalize_waits(nc):
    """This container's walrus accepts at most ONE sync wait per
    instruction; Tile emits drains/noops with many.  Split them into
    single-wait NoOps on the same engine."""
    n_split = 0
    for f in nc.m.functions:
        for blk in f.blocks:
            new = []
            for inst in blk.instructions:
                si = inst.sync_info
                if si is not None and si.on_wait and len(si.on_wait) > 1:
                    waits = list(si.on_wait)
                    for j, w in enumerate(waits[:-1]):
                        new.append(mybir.InstNoOp(
                            name=f"{inst.name}-sw{j}", engine=inst.engine,
                            sync_info=mybir.SyncInfo(on_wait=[w], on_update=[])))
                        n_split += 1
                    inst.sync_info = mybir.SyncInfo(
                        on_wait=[waits[-1]], on_update=list(si.on_update))
                new.append(inst)
            blk.instructions = new
    return n_split

F32 = mybir.dt.float32
BF16 = mybir.dt.bfloat16
ADD = mybir.AluOpType.add
MAX = mybir.AluOpType.max
IS_LT = mybir.AluOpType.is_lt
IS_EQ = mybir.AluOpType.is_equal
NEG_BIG = float(np.float32(-3.0e38))


def host_prep(inputs_np, trans_np, n_cores=8):
    """Full inputs -> per-core input maps (list of dicts)."""
    B, T, C = inputs_np.shape
    assert C == 128 and B % n_cores == 0
    bl = B // n_cores  # 16

    transT = np.ascontiguousarray(trans_np.T)  # [c, q]
    h1 = transT.astype(ml_dtypes.bfloat16)
    r1 = transT - h1.astype(np.float32)
    h2 = r1.astype(ml_dtypes.bfloat16)
    r2 = r1 - h2.astype(np.float32)
    h3 = r2.astype(ml_dtypes.bfloat16)
    assert np.all(r2 - h3.astype(np.float32) == 0.0), "bf16 split not exact"

    # TRW[b*8+ch, cl, q] = trans[q, ch*16+cl] = transT[ch*16+cl, q]
    trw = np.tile(transT.reshape(8, 16, 128)[None], (bl, 1, 1, 1))
    trw = np.ascontiguousarray(trw.reshape(128, 16 * 128), dtype=np.float32)

    iota = np.ascontiguousarray(
        np.tile(np.arange(128, dtype=np.float32)[None, :], (16, 1)))
    ident = np.eye(16, dtype=np.float32).astype(ml_dtypes.bfloat16)

    in_maps = []
    for core in range(n_cores):
        pc = inputs_np[core * bl:(core + 1) * bl]  # [16, T, 128]
        pot = pc.reshape(bl, T, 8, 16).transpose(0, 2, 1, 3)
        pot = np.ascontiguousarray(pot.reshape(128, T * 16), dtype=np.float32)
        in_maps.append({
            "pot": pot, "trw": trw,
            "g1": h1, "g2": h2, "g3": h3,
            "iota": iota, "ident": ident,
        })
    return in_maps


def build(T=2048, UF=8, UB=8, legalize=True):
    """Build the Bass program. Returns nc."""
    nc = bass.Bass()

    d_pot = nc.dram_tensor("pot", [128, T * 16], F32, kind="ExternalInput")
    d_trw = nc.dram_tensor("trw", [128, 16 * 128], F32, kind="ExternalInput")
    d_g1 = nc.dram_tensor("g1", [128, 128], BF16, kind="ExternalInput")
    d_g2 = nc.dram_tensor("g2", [128, 128], BF16, kind="ExternalInput")
    d_g3 = nc.dram_tensor("g3", [128, 128], BF16, kind="ExternalInput")
    d_iota = nc.dram_tensor("iota", [16, 128], F32, kind="ExternalInput")
    d_ident = nc.dram_tensor("ident", [16, 16], BF16, kind="ExternalInput")
    d_tags = nc.dram_tensor("tags", [16, T], F32, kind="ExternalOutput")
    d_ahist = nc.dram_tensor("ahist", [128, T * 16], F32, kind="Internal")
    # backward views
    ahist_bt = d_ahist.rearrange("(b ch) (t cl) -> b t ch cl", ch=8, cl=16)
    ahist_bch = d_ahist.rearrange("(b ch) (t cl) -> b ch t cl", ch=8, cl=16)

    with tile.TileContext(nc) as tc, ExitStack() as ctx:
        singles = ctx.enter_context(tc.tile_pool(name="singles", bufs=1))

        # ---------------- forward ----------------
        with (
            tc.tile_pool(name="potp", bufs=1) as potp,
            tc.tile_pool(name="stp", bufs=2) as stp,
            tc.tile_pool(name="scrp", bufs=2) as scrp,
        ):
            s_trw = singles.tile([128, 16, 128], F32)
            s_alpha = singles.tile([128, 128], F32)  # ALPHA_P
            s_m = singles.tile([128, 16], F32)
            nc.sync.dma_start(
                out=s_trw[:], in_=d_trw.rearrange("p (cl q) -> p cl q", cl=16))

            s_pot = potp.tile([128, T * 16], F32)
            NPC = 8  # split preload so early compute can start sooner
            for c in range(NPC):
                sl = slice(c * T * 16 // NPC, (c + 1) * T * 16 // NPC)
                nc.sync.dma_start(out=s_pot[:, sl], in_=d_pot[:, sl])

            # touch each preload chunk on DVE so the loop body carries no
            # extra DMA-queue waits (back-edge drain has limited wait slots)
            s_touch = singles.tile([128, NPC], F32)
            for c in range(NPC):
                nc.vector.tensor_copy(
                    s_touch[:, c:c + 1], s_pot[:, c * T * 16 // NPC:c * T * 16 // NPC + 1])

            # t=0 init
            st0 = stp.tile([128, UF * 16], F32, tag="stage")
            nc.vector.tensor_copy(st0[:, 0:16], s_pot[:, 0:16])
            for ch in range(8):
                mask = [(j & ~7) | ch for j in range(32)]
                nc.vector.stream_shuffle(
                    s_alpha[:, ch * 16:(ch + 1) * 16], st0[:, 0:16], mask)
            nc.sync.dma_start(out=d_ahist[:, 0:16], in_=st0[:, 0:16])

            def fwd_group(iv0, unroll):
                stage = stp.tile([128, UF * 16], F32, tag="stage")
                for k in range(unroll):
                    iv = iv0 + k * 16
                    scr = scrp.tile([128, 16, 128], F32, tag="scr")
                    alb = s_alpha[:]
                    al_bcast = bass.AP(
                        tensor=alb.tensor, offset=alb.offset,
                        ap=[list(alb.ap[0]), [0, 16], [1, 128]])
                    nc.vector.tensor_add(scr[:], s_trw[:], al_bcast)
                    nc.vector.tensor_reduce(
                        out=s_m[:], in_=scr[:], axis=mybir.AxisListType.X,
                        op=MAX)
                    ksl = slice(k * 16, (k + 1) * 16)
                    nc.vector.tensor_add(
                        stage[:, ksl], s_m[:], s_pot[:, iv:iv + 16])
                    for ch in range(8):
                        mask = [(j & ~7) | ch for j in range(32)]
                        nc.vector.stream_shuffle(
                            s_alpha[:, ch * 16:(ch + 1) * 16], stage[:, ksl],
                            mask)
                nc.sync.dma_start(
                    out=d_ahist[:, iv0:iv0 + unroll * 16],
                    in_=stage[:, 0:unroll * 16])

            ngrp, rem = divmod(T - 1, UF)
            for g in range(ngrp):
                fwd_group(16 + g * UF * 16, UF)
            if rem:
                fwd_group(16 + ngrp * UF * 16, rem)

        # ---------------- backward ----------------
        with (
            tc.tile_pool(name="abp", bufs=2) as abp,
            tc.tile_pool(name="psp", bufs=2, space="PSUM") as psp,
        ):
            s_g = [singles.tile([128, 128], BF16, name=f"g{k}") for k in range(3)]
            for sg, dg in zip(s_g, (d_g1, d_g2, d_g3)):
                nc.sync.dma_start(out=sg[:], in_=dg[:])
            s_iota = singles.tile([16, 128], F32)
            nc.sync.dma_start(out=s_iota[:], in_=d_iota[:])
            s_ident = singles.tile([16, 16], BF16)
            nc.sync.dma_start(out=s_ident[:], in_=d_ident[:])

            s_tags = singles.tile([16, T], F32)
            s_tcol = singles.tile([16, 1], F32)    # current tag per b
            s_mv = singles.tile([16, 1], F32)
            s_run = singles.tile([16, 128], F32)
            s_mask = singles.tile([16, 128], F32)
            s_ohbt = singles.tile([16, 128], BF16)
            s_oht = singles.tile([128, 16], BF16)
            s_cand = singles.tile([16, 128], F32)

            # init: tag_{T-1} = argmax(alpha_{T-1})
            s_alast = singles.tile([16, 1, 8, 16], F32)
            nc.sync.dma_start(out=s_alast[:], in_=ahist_bt[:, T - 1:T, :, :])
            al2 = s_alast[:].rearrange("b t ch cl -> b (t ch cl)")
            nc.vector.tensor_reduce(
                out=s_mv[:], in_=al2, axis=mybir.AxisListType.X, op=MAX)
            nc.vector.tensor_tensor_scan(
                out=s_run[:], data0=al2, data1=al2,
                initial=NEG_BIG, op0=MAX, op1=MAX)
            nc.vector.tensor_scalar(
                out=s_mask[:], in0=s_run[:], scalar1=s_mv[:, 0:1], scalar2=0.0,
                op0=IS_LT, op1=ADD, accum_out=s_tcol[:])
            nc.scalar.copy(s_tags[:, T - 1:T], s_tcol[:])

            def bwd_group(iv0, unroll):
                # iv0 = highest tprev in this group; covers tprev = iv0-k
                stage = abp.tile([16, UB, 8, 16], F32, tag="bstage")
                for ch in range(8):
                    nc.sync.dma_start(
                        out=stage[:, 0:unroll, ch, :],
                        in_=ahist_bt[:, iv0 - (unroll - 1):iv0 + 1, ch, :])
                st2 = stage[:].rearrange("b t ch cl -> b (t ch cl)")
                for k in range(unroll):
                    tprev_col = unroll - 1 - k
                    abf = st2[:, tprev_col * 128:(tprev_col + 1) * 128]
                    nc.vector.tensor_scalar(
                        out=s_ohbt[:], in0=s_iota[:], scalar1=s_tcol[:],
                        scalar2=None, op0=IS_EQ)
                    ohp = psp.tile([128, 16], BF16, tag="ohp")
                    nc.tensor.transpose(ohp[:], s_ohbt[:], s_ident[:])
                    nc.vector.tensor_copy(s_oht[:], ohp[:])
                    tcp = psp.tile([16, 128], F32, tag="tcp")
                    for g in range(3):
                        nc.tensor.matmul(tcp[:], s_oht[:], s_g[g][:],
                                         start=(g == 0), stop=(g == 2))
                    nc.vector.tensor_add(s_cand[:], abf, tcp[:])
                    nc.vector.tensor_reduce(
                        out=s_mv[:], in_=s_cand[:],
                        axis=mybir.AxisListType.X, op=MAX)
                    nc.vector.tensor_tensor_scan(
                        out=s_run[:], data0=s_cand[:], data1=s_cand[:],
                        initial=NEG_BIG, op0=MAX, op1=MAX)
                    nc.vector.tensor_scalar(
                        out=s_mask[:], in0=s_run[:], scalar1=s_mv[:, 0:1],
                        scalar2=0.0, op0=IS_LT, op1=ADD,
                        accum_out=s_tcol[:])
                    nc.scalar.copy(s_tags[:, iv0 - k:iv0 - k + 1],
                                   s_tcol[:])

            nbg, brem = divmod(T - 1, UB)
            start = T - 2
            for g in range(nbg):
                bwd_group(start, UB)
                start -= UB
            if brem:
                bwd_group(start, brem)

            nc.sync.dma_start(out=d_tags[:], in_=s_tags[:])

    if legalize:
        legalize_waits(nc)
    return nc



_NC_CACHE = {}


def _get_nc(T):
    if T not in _NC_CACHE:
        _NC_CACHE[T] = build(T=T, UF=8, UB=8)
    return _NC_CACHE[T]


def kernel(inputs, transitions):
    """Full-input Viterbi decode on 8 NeuronCores (data-parallel over batch)."""
    from concourse import bass_utils

    inputs = np.asarray(inputs)
    transitions = np.asarray(transitions)
    B, T, C = inputs.shape
    n_cores = 8
    in_maps = host_prep(inputs, transitions, n_cores=n_cores)
    nc = _get_nc(T)
    res = bass_utils.run_bass_kernel_spmd(
        nc, in_maps, core_ids=list(range(n_cores)))
    tags = np.concatenate([r["tags"] for r in res.results], axis=0)  # [B, T]
    return tags.astype(inputs.dtype)

